# revision 1
# baseline (speedup 1.0000x reference)
"""Trainium2 Bass kernel for nn_EquivariantGNN_GAT (2-layer GAT + linear + mean pool).

Strategy (8 NeuronCores, SPMD single program):
  - Nodes padded to 50176 = 392 blocks of 128; each core owns 49 dst-blocks
    (6272 nodes) and all edges incident (by dst) on them, host-sorted by dst.
  - Per layer, each core computes hs = x @ [W | 0 | W@a_src | W@a_dst] for its
    node shard in f32, stores the per-node row [h(128) | 1 | s_src | s_dst]
    cast to bf16 (512B rows), AllGathers the full [50176, 256] bf16 table
    into HBM, then processes its edges in chunks of 128 via dma_gather of
    hs[src] rows. int16 gather indices are handled by splitting each block's
    edges into src<32768 ("lo") and src>=32768 ("hi") halves gathered from
    offset table views; gathers are capped at 8 chunks (1024 descriptors)
    to fit the SWDGE ring.
  - Per chunk: one-hot dst matrix scaled by exp(leaky_relu(s_src + s_dst))
    built on DVE (scalar_tensor_tensor with fused accum for the s_dst
    expansion), then a single bf16 matmul accumulates numerator + softmax
    denominator ([h | 1] columns) into f32 PSUM per dst block.
  - Softmax max-subtraction is skipped (mathematically equivalent here).
  - Final: y = x3 @ Wlin + blin per block, per-graph mean pool via one-hot
    matmul accumulated in PSUM, AllReduce over cores, scale by 1/counts.

kernel(**inputs) takes the FULL problem inputs and returns the [64, 32] output.
"""
import sys

sys.path.insert(0, "/opt/trn_rl_repo")

import ml_dtypes
import numpy as np

import concourse.bass as bass
import concourse.bacc as bacc
import concourse.mybir as mybir
import concourse.tile as tile
import concourse.bass_utils as bass_utils
from concourse.bass import IndirectOffsetOnAxis
from concourse.bass_interp import get_hw_module

N = 50000
E = 1600000
H = 128
O = 32
T = 100
G = 64
P = 128
NCORES = 8
NBPC = 49              # dst blocks per core
NB = NBPC * NCORES     # 392 blocks -> 50176 padded nodes
NPAD = NB * P
SH = NBPC * P          # 6272 nodes per core
ROW = 256              # bf16 elems per hs row: [h(128) | 1 | ssrc | sdst | 0pad]
WCOL = 131             # computed columns: [W(128) | 0 | W@a_s | W@a_d]
LO = 32768             # int16 index limit; src >= LO gathered from offset view
GMAX = 8               # chunks per dma_gather (1024 descs fit the SWDGE ring)
NEG = 0.2

F32 = mybir.dt.float32
BF16 = mybir.dt.bfloat16
I32 = mybir.dt.int32
I16 = mybir.dt.int16
ALU = mybir.AluOpType
AF = mybir.ActivationFunctionType
NPBF = ml_dtypes.bfloat16


# ---------------------------------------------------------------- host prep
def _wrap16(flat):
    """dma_gather index layout: idx k -> [k%16, k//16], replicated x8."""
    n = flat.shape[0]
    assert n % 16 == 0
    w = flat.reshape(n // 16, 16).T          # [16, n//16]
    return np.tile(w, (8, 1))                 # [128, n//16]


def _prep(inputs):
    pos = np.ascontiguousarray(np.asarray(inputs["pos"], np.float32))
    z = np.asarray(inputs["z"]).astype(np.int64)
    ei = np.asarray(inputs["edge_index"]).astype(np.int64)
    batch = np.asarray(inputs["batch"]).astype(np.int64)

    loop = np.arange(N, dtype=np.int64)
    src = np.concatenate([ei[0], loop])
    dst = np.concatenate([ei[1], loop])
    order = np.argsort(dst, kind="stable")
    src_s = src[order].astype(np.int32)
    dst_s = dst[order].astype(np.int32)

    bnd = np.searchsorted(dst_s, np.arange(NB + 1) * P).astype(np.int64)

    block_lo = {}
    block_hi = {}
    for b in range(NB):
        e0, e1 = int(bnd[b]), int(bnd[b + 1])
        s = src_s[e0:e1]
        d = dst_s[e0:e1] - b * P
        m = s < LO
        block_lo[b] = (s[m], d[m])
        block_hi[b] = (s[~m] - LO, d[~m])

    nlo = np.zeros(NBPC, np.int64)
    nhi = np.zeros(NBPC, np.int64)
    for i in range(NBPC):
        for c in range(NCORES):
            b = NBPC * c + i
            nlo[i] = max(nlo[i], (len(block_lo[b][0]) + P - 1) // P)
            nhi[i] = max(nhi[i], (len(block_hi[b][0]) + P - 1) // P)
        if nlo[i] + nhi[i] == 0:
            nhi[i] = 1
    CH = nlo + nhi
    off = np.zeros(NBPC + 1, np.int64)
    off[1:] = np.cumsum(CH)
    TOTCH = int(off[-1])

    idx_w = np.zeros((NCORES, P, 8 * TOTCH), np.int16)
    dstrel = np.full((NCORES, P, TOTCH), -1.0, NPBF)
    zidx = np.zeros((NCORES, P, NBPC), np.int32)
    batchrel = np.full((NCORES, P, NBPC), -1.0, np.float32)
    pos_shard = np.zeros((NCORES, SH, 3), np.float32)

    for c in range(NCORES):
        for i in range(NBPC):
            b = NBPC * c + i
            for (s, d), n_ch, cbase in (
                (block_lo[b], int(nlo[i]), int(off[i])),
                (block_hi[b], int(nhi[i]), int(off[i]) + int(nlo[i])),
            ):
                if n_ch == 0:
                    continue
                ne = len(s)
                flat = np.zeros(n_ch * P, np.int16)
                flat[:ne] = s.astype(np.int16)
                dr = np.full(n_ch * P, -1.0, np.float32)
                dr[:ne] = d.astype(np.float32)
                idx_w[c, :, 8 * cbase:8 * (cbase + n_ch)] = _wrap16(flat)
                dstrel[c, :, cbase:cbase + n_ch] = \
                    dr.reshape(n_ch, P).T.astype(NPBF)
        n0 = SH * c
        nodes = np.arange(n0, n0 + SH)
        valid = nodes < N
        zc = np.zeros(SH, np.int32)
        zc[valid] = z[nodes[valid]].astype(np.int32)
        zidx[c] = zc.reshape(NBPC, P).T
        bc = np.full(SH, -1.0, np.float32)
        bc[valid] = batch[nodes[valid]].astype(np.float32)
        batchrel[c] = bc.reshape(NBPC, P).T
        pos_shard[c][valid] = pos[nodes[valid]]

    counts = np.bincount(batch, minlength=G).astype(np.float32)
    cinv = (1.0 / np.maximum(counts, 1.0)).astype(np.float32).reshape(G, 1)

    iota_f = np.tile(np.arange(P, dtype=np.float32), (P, 1))
    consts = dict(
        iota=np.ascontiguousarray(iota_f),
        iotab=np.ascontiguousarray(iota_f.astype(NPBF)),
        ident=np.eye(P, dtype=np.float32),
        ones1=np.ones((1, P), NPBF),
        W1=np.ascontiguousarray(np.asarray(inputs["W1"], np.float32)),
        W1T=np.ascontiguousarray(np.asarray(inputs["W1"], np.float32).T),
        a1s=np.asarray(inputs["a1_src"], np.float32).reshape(H, 1),
        a1d=np.asarray(inputs["a1_dst"], np.float32).reshape(H, 1),
        b1rep=np.ascontiguousarray(
            np.tile(np.asarray(inputs["b1"], np.float32), (P, 1))),
        W2=np.ascontiguousarray(np.asarray(inputs["W2"], np.float32)),
        W2T=np.ascontiguousarray(np.asarray(inputs["W2"], np.float32).T),
        a2s=np.asarray(inputs["a2_src"], np.float32).reshape(H, 1),
        a2d=np.asarray(inputs["a2_dst"], np.float32).reshape(H, 1),
        b2rep=np.ascontiguousarray(
            np.tile(np.asarray(inputs["b2"], np.float32), (P, 1))),
        Wlin=np.ascontiguousarray(np.asarray(inputs["Wlin"], np.float32)),
        blinrep=np.ascontiguousarray(
            np.tile(np.asarray(inputs["blin"], np.float32), (P, 1))),
        emb=np.ascontiguousarray(np.asarray(inputs["emb"], np.float32)),
        cinv=cinv,
    )

    meta = dict(nlo=tuple(int(x) for x in nlo),
                nhi=tuple(int(x) for x in nhi),
                off=tuple(int(x) for x in off),
                TOTCH=TOTCH)
    percore = dict(idx_w=idx_w, dstrel=dstrel, zidx=zidx,
                   batchrel=batchrel, pos_shard=pos_shard)
    return meta, percore, consts


# ---------------------------------------------------------------- program
def _build(meta, analysis=False, gmax=GMAX, nqueues=1, skip=(),
           scratch=16384, gbufs=3, mtact=False, bufboost=0):
    nlo = meta["nlo"]
    nhi = meta["nhi"]
    off = meta["off"]
    TOTCH = meta["TOTCH"]
    qstate = {"q": 0}

    def next_q():
        q = qstate["q"]
        qstate["q"] = (q + 1) % nqueues
        return q

    nc = bacc.Bacc("TRN2", target_bir_lowering=False, debug=False,
                   enable_asserts=False,
                   num_devices=1 if analysis else NCORES,
                   num_swdge_queues=nqueues,
                   dynamic_dma_scratch_size=scratch)

    t_idx = nc.dram_tensor("idx_w", [P, 8 * TOTCH], I16, kind="ExternalInput")
    t_dstr = nc.dram_tensor("dstrel", [P, TOTCH], BF16, kind="ExternalInput")
    t_zidx = nc.dram_tensor("zidx", [P, NBPC], I32, kind="ExternalInput")
    t_brel = nc.dram_tensor("batchrel", [P, NBPC], F32, kind="ExternalInput")
    t_pos = nc.dram_tensor("pos_shard", [SH, 3], F32, kind="ExternalInput")
    t_emb = nc.dram_tensor("emb", [T, 125], F32, kind="ExternalInput")
    t_iota = nc.dram_tensor("iota", [P, P], F32, kind="ExternalInput")
    t_iotab = nc.dram_tensor("iotab", [P, P], BF16, kind="ExternalInput")
    t_ident = nc.dram_tensor("ident", [P, P], F32, kind="ExternalInput")
    t_ones1 = nc.dram_tensor("ones1", [1, P], BF16, kind="ExternalInput")
    t_W1 = nc.dram_tensor("W1", [H, H], F32, kind="ExternalInput")
    t_W1T = nc.dram_tensor("W1T", [H, H], F32, kind="ExternalInput")
    t_a1s = nc.dram_tensor("a1s", [H, 1], F32, kind="ExternalInput")
    t_a1d = nc.dram_tensor("a1d", [H, 1], F32, kind="ExternalInput")
    t_b1rep = nc.dram_tensor("b1rep", [P, H], F32, kind="ExternalInput")
    t_W2 = nc.dram_tensor("W2", [H, H], F32, kind="ExternalInput")
    t_W2T = nc.dram_tensor("W2T", [H, H], F32, kind="ExternalInput")
    t_a2s = nc.dram_tensor("a2s", [H, 1], F32, kind="ExternalInput")
    t_a2d = nc.dram_tensor("a2d", [H, 1], F32, kind="ExternalInput")
    t_b2rep = nc.dram_tensor("b2rep", [P, H], F32, kind="ExternalInput")
    t_Wlin = nc.dram_tensor("Wlin", [H, O], F32, kind="ExternalInput")
    t_blinrep = nc.dram_tensor("blinrep", [P, O], F32, kind="ExternalInput")
    t_cinv = nc.dram_tensor("cinv", [G, 1], F32, kind="ExternalInput")
    t_out = nc.dram_tensor("out", [G, O], F32, kind="ExternalOutput")

    groups = [list(range(NCORES))]

    with tile.TileContext(nc) as tc:
        with (
            tc.tile_pool(name="const", bufs=1) as cpool,
            tc.tile_pool(name="work", bufs=3) as wpool,
            tc.tile_pool(name="gat", bufs=gbufs) as gpool,
            tc.tile_pool(name="mt", bufs=14 + bufboost) as mpool,
            tc.tile_pool(name="gp", bufs=6 + bufboost) as gppool,
            tc.tile_pool(name="sml", bufs=4 + bufboost) as spool,
            tc.tile_pool(name="ps", bufs=2, space="PSUM") as pspool,
            tc.tile_pool(name="acc", bufs=1, space="PSUM") as apool,
            tc.tile_pool(name="dram", bufs=1, space="DRAM") as dpool,
        ):
            def cload(nm, t, shape, dtype=F32):
                tl = cpool.tile(shape, dtype, name=nm, tag=nm)
                nc.sync.dma_start(tl[:, :], t.ap())
                return tl

            iota_sb = cload("iota_sb", t_iota, [P, P])
            iotab_sb = cload("iotab_sb", t_iotab, [P, P], BF16)
            ident_sb = cload("ident_sb", t_ident, [P, P])
            ones1_sb = cload("ones1_sb", t_ones1, [1, P], BF16)
            eps_sb = cpool.tile([P, 1], F32, name="eps_sb", tag="eps_sb")
            nc.vector.memset(eps_sb[:, :], 1e-38)
            b1rep_sb = cload("b1rep_sb", t_b1rep, [P, H])
            b2rep_sb = cload("b2rep_sb", t_b2rep, [P, H])
            Wlin_sb = cload("Wlin_sb", t_Wlin, [H, O])
            blinrep_sb = cload("blinrep_sb", t_blinrep, [P, O])
            cinv_sb = cload("cinv_sb", t_cinv, [G, 1])
            idx_sb = cload("idx_sb", t_idx, [P, 8 * TOTCH], I16)
            dstr_sb = cload("dstr_sb", t_dstr, [P, TOTCH], BF16)
            zidx_sb = cload("zidx_sb", t_zidx, [P, NBPC], I32)
            brel_sb = cload("brel_sb", t_brel, [P, NBPC])

            # folded weight tables We = [W | 0 | W@a_s | W@a_d]
            def fold(nm, tW, tWT, tas, tad):
                We = cpool.tile([H, WCOL], F32, name=nm, tag=nm)
                nc.sync.dma_start(We[:, 0:H], tW.ap())
                nc.vector.memset(We[:, 128:129], 0.0)
                WT_sb = cpool.tile([H, H], F32, name=nm + "_WT", tag=nm + "_WT")
                nc.sync.dma_start(WT_sb[:, :], tWT.ap())
                for col, tvec in ((129, tas), (130, tad)):
                    av = cpool.tile([H, 1], F32, name=f"{nm}_a{col}",
                                    tag=f"{nm}_a{col}")
                    nc.sync.dma_start(av[:, :], tvec.ap())
                    fps = pspool.tile([H, 1], F32, name=f"{nm}_f{col}", tag="tps")
                    nc.tensor.matmul(fps[:, :], lhsT=WT_sb[:, :], rhs=av[:, :],
                                     start=True, stop=True)
                    nc.scalar.activation(We[:, col:col + 1], fps[:, :], AF.Copy)
                return We

            W1e = fold("W1e", t_W1, t_W1T, t_a1s, t_a1d)
            W2e = fold("W2e", t_W2, t_W2T, t_a2s, t_a2d)

            hs1_sh = dpool.tile([SH, ROW], BF16, name="hs1_sh", tag="hs1_sh")
            hs1_f = dpool.tile([NPAD, ROW], BF16, name="hs1_f", tag="hs1_f",
                               addr_space="Shared")
            hs2_sh = dpool.tile([SH, ROW], BF16, name="hs2_sh", tag="hs2_sh")
            hs2_f = dpool.tile([NPAD, ROW], BF16, name="hs2_f", tag="hs2_f",
                               addr_space="Shared")
            pool_in = dpool.tile([G, O], F32, name="pool_in", tag="pool_in")
            pool_out = dpool.tile([G, O], F32, name="pool_out", tag="pool_out",
                                  addr_space="Shared")

            # store [h | 1 | ssrc | sdst] (f32 psum) as bf16 row, zero pad
            def store_hs(hs_ps, hs_dram, i):
                hs_sb = wpool.tile([P, ROW], BF16, name="hs_sb", tag="hs_sb")
                nc.scalar.activation(hs_sb[:, 0:WCOL], hs_ps[:, :], AF.Copy)
                nc.vector.memset(hs_sb[:, 128:129], 1.0)
                nc.vector.memset(hs_sb[:, WCOL:ROW], 0.0)
                nc.sync.dma_start(hs_dram[i * P:(i + 1) * P, :], hs_sb[:, :])

            # ---------------- stage A: hs1 for own shard ----------------
            for i in range(NBPC):
                x1 = wpool.tile([P, H], F32, name="x1", tag="x1")
                nc.gpsimd.indirect_dma_start(
                    out=x1[:, 3:128], out_offset=None, in_=t_emb.ap(),
                    in_offset=IndirectOffsetOnAxis(ap=zidx_sb[:, i:i + 1], axis=0))
                nc.sync.dma_start(x1[:, 0:3], t_pos.ap()[i * P:(i + 1) * P, :])
                xt_ps = pspool.tile([P, P], F32, name="xt_ps", tag="tps")
                nc.tensor.transpose(xt_ps[:, :], x1[:, :], ident_sb[:, :])
                x1t = wpool.tile([P, P], F32, name="x1t", tag="x1t")
                nc.scalar.activation(x1t[:, :], xt_ps[:, :], AF.Copy)
                hs_ps = pspool.tile([P, WCOL], F32, name="hs_ps", tag="hsps")
                nc.tensor.matmul(hs_ps[:, :], lhsT=x1t[:, :], rhs=W1e[:, :],
                                 start=True, stop=True)
                store_hs(hs_ps, hs1_sh, i)

            if analysis:
                nc.sync.dma_start(hs1_f[0:SH, :], hs1_sh[:, :])
            else:
                nc.gpsimd.collective_compute(
                    "AllGather", ALU.bypass, groups,
                    ins=[hs1_sh[:, :]], outs=[hs1_f[:, :]])

            # ---------------- edge phase ----------------
            def edge_phase(hs_f, hs_sh_d, post_block):
                for i in range(NBPC):
                    # block prep: s_dst row replicated across partitions
                    sd_row = spool.tile([1, P], BF16, name="sd_row",
                                        tag="sd_row")
                    nc.sync.dma_start(
                        sd_row[:, :],
                        hs_sh_d[i * P:(i + 1) * P, 130:131].transpose([1, 0]))
                    sd_ps = pspool.tile([P, P], F32, name="sd_ps", tag="tps")
                    nc.tensor.matmul(sd_ps[:, :], lhsT=ones1_sb[:, :],
                                     rhs=sd_row[:, :], start=True, stop=True)
                    sdst_rep = wpool.tile([P, P], BF16, name="sdst_rep",
                                          tag="sdst_rep")
                    nc.scalar.activation(sdst_rep[:, :], sd_ps[:, :], AF.Copy)
                    num_ps = pspool.tile([P, 129], F32, name="num_ps",
                                         tag="numps")

                    halves = []
                    if nlo[i] > 0:
                        halves.append((int(nlo[i]), int(off[i]),
                                       hs_f[0:LO, :]))
                    if nhi[i] > 0:
                        halves.append((int(nhi[i]), int(off[i]) + int(nlo[i]),
                                       hs_f[LO:NPAD, :]))

                    nch = int(nlo[i]) + int(nhi[i])
                    # emit all gathers for this block first (prefetch)
                    work = []
                    done = 0
                    for n, cbase, table in halves:
                        for s0 in range(0, n, gmax):
                            sn = min(gmax, n - s0)
                            cb = cbase + s0
                            Gt = gpool.tile([P, sn * ROW], BF16, name="Gt",
                                            tag="Gt")
                            nc.gpsimd.dma_gather(
                                out_ap=Gt.rearrange("p (c s) -> p c s", s=ROW),
                                in_ap=table,
                                idxs_ap=idx_sb[:, 8 * cb:8 * (cb + sn)],
                                num_idxs=sn * P,
                                num_idxs_reg=sn * P,
                                elem_size=ROW,
                                queue_num=next_q(),
                            )
                            work.append((Gt, sn, cb, done + s0))
                        done += n
                    for Gt, sn, cb, base in work:
                        SD = spool.tile([P, sn], F32, name="SD", tag="SD")
                        for jj in range(sn):
                            junk = mpool.tile([P, P], BF16, name="junk",
                                              tag="junk")
                            nc.vector.scalar_tensor_tensor(
                                out=junk[:, :], in0=iotab_sb[:, :],
                                scalar=dstr_sb[:, cb + jj:cb + jj + 1],
                                in1=sdst_rep[:, :],
                                op0=ALU.is_equal, op1=ALU.mult,
                                accum_out=SD[:, jj:jj + 1])
                        ssrc = Gt.rearrange("p (c s) -> p c s",
                                            s=ROW)[:, :, 129:130].squeeze(2)
                        Q = spool.tile([P, sn], F32, name="Q", tag="Q")
                        nc.vector.tensor_tensor(out=Q[:, :], in0=SD[:, :],
                                                in1=ssrc, op=ALU.add)
                        V = spool.tile([P, sn], F32, name="V", tag="V")
                        nc.vector.scalar_tensor_tensor(
                            out=V[:, :], in0=Q[:, :], scalar=NEG,
                            in1=Q[:, :], op0=ALU.mult, op1=ALU.max)
                        Pe = spool.tile([P, sn], BF16, name="Pe", tag="Pe")
                        nc.scalar.activation(Pe[:, :], V[:, :], AF.Exp)
                        for jj in range(sn):
                            MT = mpool.tile([P, P], BF16, name="MT", tag="MT")
                            nc.vector.scalar_tensor_tensor(
                                out=MT[:, :], in0=iotab_sb[:, :],
                                scalar=dstr_sb[:, cb + jj:cb + jj + 1],
                                in1=Pe[:, jj:jj + 1].to_broadcast([P, P]),
                                op0=ALU.is_equal, op1=ALU.mult)
                            if "mm" not in skip or base + jj == 0:
                                nc.tensor.matmul(
                                    num_ps[:, :], lhsT=MT[:, :],
                                    rhs=Gt[:, jj * ROW:jj * ROW + 129],
                                    start=(base + jj == 0),
                                    stop=(base + jj == nch - 1)
                                    if "mm" not in skip else True)
                    post_block(i, num_ps)

            # common post-block epilogue: x = elu(num/den + b)
            def finish_x(num_ps, brep_sb):
                den = spool.tile([P, 1], F32, name="den", tag="den")
                nc.vector.tensor_scalar(out=den[:, :], in0=num_ps[:, 128:129],
                                        scalar1=1e-30, scalar2=None, op0=ALU.max)
                rec = spool.tile([P, 1], F32, name="rec", tag="rec")
                nc.vector.reciprocal(rec[:, :], den[:, :])
                xp = wpool.tile([P, H], F32, name="xp", tag="xp")
                nc.vector.scalar_tensor_tensor(
                    out=xp[:, :], in0=num_ps[:, 0:128], scalar=rec[:, :],
                    in1=brep_sb[:, :], op0=ALU.mult, op1=ALU.add)
                xm = wpool.tile([P, H], F32, name="xm", tag="xm")
                nc.vector.tensor_scalar(out=xm[:, :], in0=xp[:, :], scalar1=0.0,
                                        scalar2=None, op0=ALU.min)
                xe = wpool.tile([P, H], F32, name="xe", tag="xe")
                nc.scalar.activation(xe[:, :], xm[:, :], AF.Exp)
                xr = wpool.tile([P, H], F32, name="xr", tag="xr")
                nc.vector.tensor_scalar(out=xr[:, :], in0=xp[:, :], scalar1=0.0,
                                        scalar2=None, op0=ALU.max)
                x2 = wpool.tile([P, H], F32, name="x2", tag="x2")
                nc.vector.scalar_tensor_tensor(
                    out=x2[:, :], in0=xe[:, :], scalar=-1.0, in1=xr[:, :],
                    op0=ALU.add, op1=ALU.add)
                return x2

            # layer-1 post: x2 -> hs2 shard rows
            def post1(i, num_ps):
                x2 = finish_x(num_ps, b1rep_sb)
                xt_ps = pspool.tile([P, P], F32, name="x2t_ps", tag="tps")
                nc.tensor.transpose(xt_ps[:, :], x2[:, :], ident_sb[:, :])
                x2t = wpool.tile([P, P], F32, name="x2t", tag="x2t")
                nc.scalar.activation(x2t[:, :], xt_ps[:, :], AF.Copy)
                hs_ps = pspool.tile([P, WCOL], F32, name="hs2_ps", tag="hsps")
                nc.tensor.matmul(hs_ps[:, :], lhsT=x2t[:, :], rhs=W2e[:, :],
                                 start=True, stop=True)
                store_hs(hs_ps, hs2_sh, i)

            edge_phase(hs1_f, hs1_sh, post1)

            if analysis:
                nc.sync.dma_start(hs2_f[0:SH, :], hs2_sh[:, :])
            else:
                nc.gpsimd.collective_compute(
                    "AllGather", ALU.bypass, groups,
                    ins=[hs2_sh[:, :]], outs=[hs2_f[:, :]])

            # layer-2 post: y = x3 @ Wlin + blin; pool matmul accumulate
            pool_ps = apool.tile([G, O], F32, name="pool_ps", tag="poolps")

            def post2(i, num_ps):
                x3 = finish_x(num_ps, b2rep_sb)
                xt_ps = pspool.tile([P, P], F32, name="x3t_ps", tag="tps")
                nc.tensor.transpose(xt_ps[:, :], x3[:, :], ident_sb[:, :])
                x3t = wpool.tile([P, P], F32, name="x3t", tag="x2t")
                nc.scalar.activation(x3t[:, :], xt_ps[:, :], AF.Copy)
                y_ps = pspool.tile([P, O], F32, name="y_ps", tag="hsps")
                nc.tensor.matmul(y_ps[:, :], lhsT=x3t[:, :], rhs=Wlin_sb[:, :],
                                 start=True, stop=True)
                y_sb = wpool.tile([P, O], F32, name="y_sb", tag="y_sb")
                nc.vector.tensor_tensor(out=y_sb[:, :], in0=y_ps[:, :],
                                        in1=blinrep_sb[:, :], op=ALU.add)
                Mg = wpool.tile([P, G], F32, name="Mg", tag="Mg")
                nc.vector.tensor_scalar(out=Mg[:, :], in0=iota_sb[:, 0:G],
                                        scalar1=brel_sb[:, i:i + 1],
                                        scalar2=None, op0=ALU.is_equal)
                nc.tensor.matmul(pool_ps[:, :], lhsT=Mg[:, :], rhs=y_sb[:, :],
                                 start=(i == 0), stop=(i == NBPC - 1))

            edge_phase(hs2_f, hs2_sh, post2)

            # ---------------- final reduce ----------------
            pool_sb = spool.tile([G, O], F32, name="pool_sb", tag="pool_sb")
            nc.scalar.activation(pool_sb[:, :], pool_ps[:, :], AF.Copy)
            nc.sync.dma_start(pool_in[:, :], pool_sb[:, :])
            if analysis:
                nc.sync.dma_start(pool_out[:, :], pool_in[:, :])
            else:
                nc.gpsimd.collective_compute(
                    "AllReduce", ALU.add, groups,
                    ins=[pool_in[:, :]], outs=[pool_out[:, :]])
            red_sb = spool.tile([G, O], F32, name="red_sb", tag="red_sb")
            nc.sync.dma_start(red_sb[:, :], pool_out[:, :])
            fin_sb = spool.tile([G, O], F32, name="fin_sb", tag="fin_sb")
            nc.vector.tensor_scalar(out=fin_sb[:, :], in0=red_sb[:, :],
                                    scalar1=cinv_sb[:, :], scalar2=None,
                                    op0=ALU.mult)
            nc.sync.dma_start(t_out.ap(), fin_sb[:, :])

    nc.compile()
    nc.m = get_hw_module(nc.m)
    return nc


_CACHE = {}


def _get_nc(meta):
    key = (meta["TOTCH"], meta["nlo"], meta["nhi"])
    if key not in _CACHE:
        _CACHE[key] = _build(meta)
    return _CACHE[key]


def run(inputs, trace=False, **kw):
    meta, percore, consts = _prep(inputs)
    nc = _get_nc(meta)
    in_maps = []
    for c in range(NCORES):
        m = dict(consts)
        m["idx_w"] = np.ascontiguousarray(percore["idx_w"][c])
        m["dstrel"] = np.ascontiguousarray(percore["dstrel"][c])
        m["zidx"] = np.ascontiguousarray(percore["zidx"][c])
        m["batchrel"] = np.ascontiguousarray(percore["batchrel"][c])
        m["pos_shard"] = np.ascontiguousarray(percore["pos_shard"][c])
        in_maps.append(m)
    res = bass_utils.run_bass_kernel_spmd(
        nc, in_maps, core_ids=list(range(NCORES)), trace=trace, **kw)
    return res


def kernel(**inputs):
    res = run(inputs, trace=False)
    return res.results[0]["out"]



# revision 6
# speedup vs baseline: 1.2545x; 1.2545x over previous
"""Trainium2 Bass kernel for nn_EquivariantGNN_GAT (2-layer GAT + linear + mean pool).

Strategy (8 NeuronCores, SPMD single program):
  - Nodes padded to 50176 = 392 blocks of 128; each core owns 49 dst-blocks
    (6272 nodes) and all edges incident (by dst) on them, host-sorted by dst.
  - Per layer, each core computes hs = x @ [W | 0 | W@a_src | W@a_dst] for its
    node shard in f32, stores the per-node row [h(128) | 1 | s_src | s_dst]
    cast to bf16 (512B rows), AllGathers the full [50176, 256] bf16 table
    into HBM, then processes its edges in chunks of 128 via dma_gather of
    hs[src] rows. int16 gather indices are handled by splitting each block's
    edges into src<32768 ("lo") and src>=32768 ("hi") halves gathered from
    offset table views; gathers are capped at 8 chunks (1024 descriptors).
  - Per chunk: the dst one-hot matrix is HOST-PRECOMPUTED (graph-static) and
    DMA'd from HBM via static HWDGE (no DVE build, no SWDGE descgen). Scores:
    s_dst per edge via one STT accumulate against the one-hot; Pe =
    exp(leaky_relu(s_src + s_dst)) with the exp on ScalarE. The matmul rhs is
    the gathered row scaled by Pe on ScalarE (per-partition activation scale),
    so its "1" column carries Pe for the softmax denominator. lhsT is the pure
    one-hot; a single bf16 matmul accumulates numerator + denominator into
    f32 PSUM per dst block.
  - Softmax max-subtraction is skipped (mathematically equivalent here).
  - ELU epilogue runs on ScalarE (relu/exp) + one DVE combine, avoiding fp32
    2-port DVE ops that contend with SWDGE descriptor rings.
  - Layer-1 inputs x1 = [pos | emb[z]] and the per-graph pool one-hots are
    host-precomputed; stage A is one matmul per block.
  - Final: y = x3 @ Wlin + blin per block, per-graph mean pool via one-hot
    matmul accumulated in PSUM, AllReduce over cores, scale by 1/counts.

kernel(**inputs) takes the FULL problem inputs and returns the [64, 32] output.
"""
import sys

sys.path.insert(0, "/opt/trn_rl_repo")

import ml_dtypes
import numpy as np

import concourse.bass as bass
import concourse.bacc as bacc
import concourse.mybir as mybir
import concourse.tile as tile
import concourse.bass_utils as bass_utils
from concourse.bass_interp import get_hw_module

N = 50000
E = 1600000
H = 128
O = 32
T = 100
G = 64
P = 128
NCORES = 8
NBPC = 49              # dst blocks per core
NB = NBPC * NCORES     # 392 blocks -> 50176 padded nodes
NPAD = NB * P
SH = NBPC * P          # 6272 nodes per core
ROW = 256              # bf16 elems per hs row: [h(128) | 1 | ssrc | sdst | 0pad]
WCOL = 131             # computed columns: [W(128) | 0 | W@a_s | W@a_d]
LO = 32768             # int16 index limit; src >= LO gathered from offset view
GMAX = 8               # chunks per dma_gather (1024 descs fit the SWDGE ring)
NEG = 0.2

F32 = mybir.dt.float32
BF16 = mybir.dt.bfloat16
I32 = mybir.dt.int32
I16 = mybir.dt.int16
ALU = mybir.AluOpType
AF = mybir.ActivationFunctionType
NPBF = ml_dtypes.bfloat16


# ---------------------------------------------------------------- host prep
def _wrap16(flat):
    """dma_gather index layout: idx k -> [k%16, k//16], replicated x8."""
    n = flat.shape[0]
    assert n % 16 == 0
    w = flat.reshape(n // 16, 16).T          # [16, n//16]
    return np.tile(w, (8, 1))                 # [128, n//16]


def _prep(inputs):
    pos = np.ascontiguousarray(np.asarray(inputs["pos"], np.float32))
    z = np.asarray(inputs["z"]).astype(np.int64)
    ei = np.asarray(inputs["edge_index"]).astype(np.int64)
    batch = np.asarray(inputs["batch"]).astype(np.int64)
    emb = np.asarray(inputs["emb"], np.float32)

    loop = np.arange(N, dtype=np.int64)
    src = np.concatenate([ei[0], loop])
    dst = np.concatenate([ei[1], loop])
    order = np.argsort(dst, kind="stable")
    src_s = src[order].astype(np.int32)
    dst_s = dst[order].astype(np.int32)

    bnd = np.searchsorted(dst_s, np.arange(NB + 1) * P).astype(np.int64)

    block_lo = {}
    block_hi = {}
    for b in range(NB):
        e0, e1 = int(bnd[b]), int(bnd[b + 1])
        s = src_s[e0:e1]
        d = dst_s[e0:e1] - b * P
        m = s < LO
        block_lo[b] = (s[m], d[m])
        block_hi[b] = (s[~m] - LO, d[~m])

    nlo = np.zeros(NBPC, np.int64)
    nhi = np.zeros(NBPC, np.int64)
    for i in range(NBPC):
        for c in range(NCORES):
            b = NBPC * c + i
            nlo[i] = max(nlo[i], (len(block_lo[b][0]) + P - 1) // P)
            nhi[i] = max(nhi[i], (len(block_hi[b][0]) + P - 1) // P)
        if nlo[i] + nhi[i] == 0:
            nhi[i] = 1
    CH = nlo + nhi
    off = np.zeros(NBPC + 1, np.int64)
    off[1:] = np.cumsum(CH)
    TOTCH = int(off[-1])

    idx_w = np.zeros((NCORES, P, 8 * TOTCH), np.int16)
    oh = np.zeros((NCORES, P, TOTCH * P), NPBF)
    mgoh = np.zeros((NCORES, P, NBPC * G), np.float32)
    x1t = np.zeros((NCORES, P, SH), np.float32)

    x1_full = np.concatenate([pos, emb[z]], axis=-1).astype(np.float32)  # [N,128]

    for c in range(NCORES):
        for i in range(NBPC):
            b = NBPC * c + i
            for (s, d), n_ch, cbase in (
                (block_lo[b], int(nlo[i]), int(off[i])),
                (block_hi[b], int(nhi[i]), int(off[i]) + int(nlo[i])),
            ):
                if n_ch == 0:
                    continue
                ne = len(s)
                flat = np.zeros(n_ch * P, np.int16)
                flat[:ne] = s.astype(np.int16)
                idx_w[c, :, 8 * cbase:8 * (cbase + n_ch)] = _wrap16(flat)
                if ne:
                    e = np.arange(ne)
                    col = (cbase + e // P) * P + d[:ne]
                    oh[c, e % P, col] = 1.0
        n0 = SH * c
        nodes = np.arange(n0, n0 + SH)
        valid = nodes < N
        bc = np.full(SH, -1, np.int64)
        bc[valid] = batch[nodes[valid]]
        for i in range(NBPC):
            blk = bc[i * P:(i + 1) * P]
            vm = blk >= 0
            mgoh[c, np.arange(P)[vm], i * G + blk[vm]] = 1.0
        x1t[c][:, valid[np.arange(SH)]] = x1_full[nodes[valid]].T

    counts = np.bincount(batch, minlength=G).astype(np.float32)
    cinv = (1.0 / np.maximum(counts, 1.0)).astype(np.float32).reshape(G, 1)

    consts = dict(
        ones1=np.ones((1, P), NPBF),
        ident=np.eye(P, dtype=np.float32),
        W1=np.ascontiguousarray(np.asarray(inputs["W1"], np.float32)),
        W1T=np.ascontiguousarray(np.asarray(inputs["W1"], np.float32).T),
        a1s=np.asarray(inputs["a1_src"], np.float32).reshape(H, 1),
        a1d=np.asarray(inputs["a1_dst"], np.float32).reshape(H, 1),
        b1rep=np.ascontiguousarray(
            np.tile(np.asarray(inputs["b1"], np.float32), (P, 1))),
        W2=np.ascontiguousarray(np.asarray(inputs["W2"], np.float32)),
        W2T=np.ascontiguousarray(np.asarray(inputs["W2"], np.float32).T),
        a2s=np.asarray(inputs["a2_src"], np.float32).reshape(H, 1),
        a2d=np.asarray(inputs["a2_dst"], np.float32).reshape(H, 1),
        b2rep=np.ascontiguousarray(
            np.tile(np.asarray(inputs["b2"], np.float32), (P, 1))),
        Wlin=np.ascontiguousarray(np.asarray(inputs["Wlin"], np.float32)),
        blinrep=np.ascontiguousarray(
            np.tile(np.asarray(inputs["blin"], np.float32), (P, 1))),
        cinv=cinv,
    )

    meta = dict(nlo=tuple(int(x) for x in nlo),
                nhi=tuple(int(x) for x in nhi),
                off=tuple(int(x) for x in off),
                TOTCH=TOTCH)
    percore = dict(idx_w=idx_w, oh=oh, mgoh=mgoh, x1t=x1t)
    return meta, percore, consts


# ---------------------------------------------------------------- program
def _build(meta, analysis=False, gmax=GMAX, nqueues=2, skip=(),
           scratch=16384, gbufs=3, bufboost=0):
    nlo = meta["nlo"]
    nhi = meta["nhi"]
    off = meta["off"]
    TOTCH = meta["TOTCH"]
    qstate = {"q": 0}

    def next_q():
        q = qstate["q"]
        qstate["q"] = (q + 1) % nqueues
        return q

    nc = bacc.Bacc("TRN2", target_bir_lowering=False, debug=False,
                   enable_asserts=False,
                   num_devices=1 if analysis else NCORES,
                   num_swdge_queues=nqueues,
                   dynamic_dma_scratch_size=scratch)

    t_idx = nc.dram_tensor("idx_w", [P, 8 * TOTCH], I16, kind="ExternalInput")
    t_oh = nc.dram_tensor("oh", [P, TOTCH * P], BF16, kind="ExternalInput")
    t_mgoh = nc.dram_tensor("mgoh", [P, NBPC * G], F32, kind="ExternalInput")
    t_x1t = nc.dram_tensor("x1t", [P, SH], F32, kind="ExternalInput")
    t_ones1 = nc.dram_tensor("ones1", [1, P], BF16, kind="ExternalInput")
    t_ident = nc.dram_tensor("ident", [P, P], F32, kind="ExternalInput")
    t_W1 = nc.dram_tensor("W1", [H, H], F32, kind="ExternalInput")
    t_W1T = nc.dram_tensor("W1T", [H, H], F32, kind="ExternalInput")
    t_a1s = nc.dram_tensor("a1s", [H, 1], F32, kind="ExternalInput")
    t_a1d = nc.dram_tensor("a1d", [H, 1], F32, kind="ExternalInput")
    t_b1rep = nc.dram_tensor("b1rep", [P, H], F32, kind="ExternalInput")
    t_W2 = nc.dram_tensor("W2", [H, H], F32, kind="ExternalInput")
    t_W2T = nc.dram_tensor("W2T", [H, H], F32, kind="ExternalInput")
    t_a2s = nc.dram_tensor("a2s", [H, 1], F32, kind="ExternalInput")
    t_a2d = nc.dram_tensor("a2d", [H, 1], F32, kind="ExternalInput")
    t_b2rep = nc.dram_tensor("b2rep", [P, H], F32, kind="ExternalInput")
    t_Wlin = nc.dram_tensor("Wlin", [H, O], F32, kind="ExternalInput")
    t_blinrep = nc.dram_tensor("blinrep", [P, O], F32, kind="ExternalInput")
    t_cinv = nc.dram_tensor("cinv", [G, 1], F32, kind="ExternalInput")
    t_out = nc.dram_tensor("out", [G, O], F32, kind="ExternalOutput")

    groups = [list(range(NCORES))]

    with tile.TileContext(nc) as tc:
        with (
            tc.tile_pool(name="const", bufs=1) as cpool,
            tc.tile_pool(name="work", bufs=3) as wpool,
            tc.tile_pool(name="gat", bufs=gbufs) as gpool,
            tc.tile_pool(name="ohp", bufs=gbufs) as ohpool,
            tc.tile_pool(name="mt", bufs=14 + bufboost) as mpool,
            tc.tile_pool(name="sml", bufs=4 + bufboost) as spool,
            tc.tile_pool(name="ps", bufs=2, space="PSUM") as pspool,
            tc.tile_pool(name="acc", bufs=1, space="PSUM") as apool,
            tc.tile_pool(name="dram", bufs=1, space="DRAM") as dpool,
        ):
            def cload(nm, t, shape, dtype=F32):
                tl = cpool.tile(shape, dtype, name=nm, tag=nm)
                nc.sync.dma_start(tl[:, :], t.ap())
                return tl

            ones1_sb = cload("ones1_sb", t_ones1, [1, P], BF16)
            ident_sb = cload("ident_sb", t_ident, [P, P])
            b1rep_sb = cload("b1rep_sb", t_b1rep, [P, H])
            b2rep_sb = cload("b2rep_sb", t_b2rep, [P, H])
            Wlin_sb = cload("Wlin_sb", t_Wlin, [H, O])
            blinrep_sb = cload("blinrep_sb", t_blinrep, [P, O])
            cinv_sb = cload("cinv_sb", t_cinv, [G, 1])
            idx_sb = cload("idx_sb", t_idx, [P, 8 * TOTCH], I16)
            mgoh_sb = cload("mgoh_sb", t_mgoh, [P, NBPC * G])

            # folded weight tables We = [W | 0 | W@a_s | W@a_d]
            def fold(nm, tW, tWT, tas, tad):
                We = cpool.tile([H, WCOL], F32, name=nm, tag=nm)
                nc.sync.dma_start(We[:, 0:H], tW.ap())
                nc.vector.memset(We[:, 128:129], 0.0)
                WT_sb = cpool.tile([H, H], F32, name=nm + "_WT", tag=nm + "_WT")
                nc.sync.dma_start(WT_sb[:, :], tWT.ap())
                for col, tvec in ((129, tas), (130, tad)):
                    av = cpool.tile([H, 1], F32, name=f"{nm}_a{col}",
                                    tag=f"{nm}_a{col}")
                    nc.sync.dma_start(av[:, :], tvec.ap())
                    fps = pspool.tile([H, 1], F32, name=f"{nm}_f{col}", tag="tps")
                    nc.tensor.matmul(fps[:, :], lhsT=WT_sb[:, :], rhs=av[:, :],
                                     start=True, stop=True)
                    nc.scalar.activation(We[:, col:col + 1], fps[:, :], AF.Copy)
                return We

            W1e = fold("W1e", t_W1, t_W1T, t_a1s, t_a1d)
            W2e = fold("W2e", t_W2, t_W2T, t_a2s, t_a2d)

            hs1_sh = dpool.tile([SH, ROW], BF16, name="hs1_sh", tag="hs1_sh")
            hs1_f = dpool.tile([NPAD, ROW], BF16, name="hs1_f", tag="hs1_f",
                               addr_space="Shared")
            hs2_sh = dpool.tile([SH, ROW], BF16, name="hs2_sh", tag="hs2_sh")
            hs2_f = dpool.tile([NPAD, ROW], BF16, name="hs2_f", tag="hs2_f",
                               addr_space="Shared")
            pool_in = dpool.tile([G, O], F32, name="pool_in", tag="pool_in")
            pool_out = dpool.tile([G, O], F32, name="pool_out", tag="pool_out",
                                  addr_space="Shared")

            # store [h | 1 | ssrc | sdst] (f32 psum) as bf16 row, zero pad
            def store_hs(hs_ps, hs_dram, i):
                hs_sb = wpool.tile([P, ROW], BF16, name="hs_sb", tag="hs_sb")
                nc.scalar.activation(hs_sb[:, 0:WCOL], hs_ps[:, :], AF.Copy)
                nc.vector.memset(hs_sb[:, 128:129], 1.0)
                nc.vector.memset(hs_sb[:, WCOL:ROW], 0.0)
                nc.sync.dma_start(hs_dram[i * P:(i + 1) * P, :], hs_sb[:, :])

            # ---------------- stage A: hs1 for own shard ----------------
            for i in range(NBPC):
                x1t = wpool.tile([P, P], F32, name="x1t", tag="x1t")
                nc.sync.dma_start(x1t[:, :], t_x1t.ap()[:, i * P:(i + 1) * P])
                hs_ps = pspool.tile([P, WCOL], F32, name="hs_ps", tag="hsps")
                nc.tensor.matmul(hs_ps[:, :], lhsT=x1t[:, :], rhs=W1e[:, :],
                                 start=True, stop=True)
                store_hs(hs_ps, hs1_sh, i)

            if analysis:
                nc.sync.dma_start(hs1_f[0:SH, :], hs1_sh[:, :])
            else:
                nc.gpsimd.collective_compute(
                    "AllGather", ALU.bypass, groups,
                    ins=[hs1_sh[:, :]], outs=[hs1_f[:, :]])

            # ---------------- edge phase ----------------
            def edge_phase(hs_f, hs_sh_d, post_block):
                for i in range(NBPC):
                    # block prep: s_dst row replicated across partitions
                    sd_row = spool.tile([1, P], BF16, name="sd_row",
                                        tag="sd_row")
                    nc.sync.dma_start(
                        sd_row[:, :],
                        hs_sh_d[i * P:(i + 1) * P, 130:131].transpose([1, 0]))
                    sd_ps = pspool.tile([P, P], F32, name="sd_ps", tag="tps")
                    nc.tensor.matmul(sd_ps[:, :], lhsT=ones1_sb[:, :],
                                     rhs=sd_row[:, :], start=True, stop=True)
                    sdst_rep = wpool.tile([P, P], BF16, name="sdst_rep",
                                          tag="sdst_rep")
                    nc.scalar.activation(sdst_rep[:, :], sd_ps[:, :], AF.Copy)
                    num_ps = pspool.tile([P, 129], F32, name="num_ps",
                                         tag="numps")

                    halves = []
                    if nlo[i] > 0:
                        halves.append((int(nlo[i]), int(off[i]),
                                       hs_f[0:LO, :]))
                    if nhi[i] > 0:
                        halves.append((int(nhi[i]), int(off[i]) + int(nlo[i]),
                                       hs_f[LO:NPAD, :]))

                    nch = int(nlo[i]) + int(nhi[i])
                    # emit all gathers for this block first (prefetch)
                    work = []
                    done = 0
                    for n, cbase, table in halves:
                        for s0 in range(0, n, gmax):
                            sn = min(gmax, n - s0)
                            cb = cbase + s0
                            Gt = gpool.tile([P, sn * ROW], BF16, name="Gt",
                                            tag="Gt")
                            nc.gpsimd.dma_gather(
                                out_ap=Gt.rearrange("p (c s) -> p c s", s=ROW),
                                in_ap=table,
                                idxs_ap=idx_sb[:, 8 * cb:8 * (cb + sn)],
                                num_idxs=sn * P,
                                num_idxs_reg=sn * P,
                                elem_size=ROW,
                                queue_num=next_q(),
                            )
                            ohg = ohpool.tile([P, sn * P], BF16, name="ohg",
                                              tag="ohg")
                            nc.sync.dma_start(
                                ohg[:, :],
                                t_oh.ap()[:, P * cb:P * (cb + sn)])
                            work.append((Gt, ohg, sn, cb, done + s0))
                        done += n
                    for Gt, ohg, sn, cb, base in work:
                        SD = spool.tile([P, sn], F32, name="SD", tag="SD")
                        for jj in range(sn):
                            junk = mpool.tile([P, P], BF16, name="junk",
                                              tag="junk")
                            nc.vector.scalar_tensor_tensor(
                                out=junk[:, :],
                                in0=ohg[:, jj * P:(jj + 1) * P],
                                scalar=1.0,
                                in1=sdst_rep[:, :],
                                op0=ALU.mult, op1=ALU.mult,
                                accum_out=SD[:, jj:jj + 1])
                        ssrc = Gt.rearrange("p (c s) -> p c s",
                                            s=ROW)[:, :, 129:130].squeeze(2)
                        Q = spool.tile([P, sn], F32, name="Q", tag="Q")
                        nc.vector.tensor_tensor(out=Q[:, :], in0=SD[:, :],
                                                in1=ssrc, op=ALU.add)
                        V = spool.tile([P, sn], F32, name="V", tag="V")
                        nc.vector.scalar_tensor_tensor(
                            out=V[:, :], in0=Q[:, :], scalar=NEG,
                            in1=Q[:, :], op0=ALU.mult, op1=ALU.max)
                        Pe = spool.tile([P, sn], F32, name="Pe", tag="Pe")
                        nc.scalar.activation(Pe[:, :], V[:, :], AF.Exp)
                        for jj in range(sn):
                            peg = mpool.tile([P, 129], BF16, name="peg",
                                             tag="peg")
                            nc.scalar.mul(peg[:, :],
                                          Gt[:, jj * ROW:jj * ROW + 129],
                                          Pe[:, jj:jj + 1])
                            if "mm" not in skip or base + jj == 0:
                                nc.tensor.matmul(
                                    num_ps[:, :],
                                    lhsT=ohg[:, jj * P:(jj + 1) * P],
                                    rhs=peg[:, :],
                                    start=(base + jj == 0),
                                    stop=(base + jj == nch - 1)
                                    if "mm" not in skip else True)
                    post_block(i, num_ps)

            # common post-block epilogue: x = elu(num/den + b)
            # elu(x) = relu(x) + exp(min(x, 0)) - 1; min(x,0) = -relu(-x).
            # relu/exp on ScalarE (fp32 2-port DVE ops contend with SWDGE
            # descriptor rings), one DVE combine.
            def finish_x(num_ps, brep_sb):
                den = spool.tile([P, 1], F32, name="den", tag="den")
                nc.vector.tensor_scalar(out=den[:, :], in0=num_ps[:, 128:129],
                                        scalar1=1e-30, scalar2=None, op0=ALU.max)
                rec = spool.tile([P, 1], F32, name="rec", tag="rec")
                nc.vector.reciprocal(rec[:, :], den[:, :])
                xp = wpool.tile([P, H], F32, name="xp", tag="xp")
                nc.vector.scalar_tensor_tensor(
                    out=xp[:, :], in0=num_ps[:, 0:128], scalar=rec[:, :],
                    in1=brep_sb[:, :], op0=ALU.mult, op1=ALU.add)
                xr = wpool.tile([P, H], F32, name="xr", tag="xr")
                nc.scalar.activation(xr[:, :], xp[:, :], AF.Relu)
                xmn = wpool.tile([P, H], F32, name="xmn", tag="xmn")
                nc.scalar.activation(xmn[:, :], xp[:, :], AF.Relu, scale=-1.0)
                xe = wpool.tile([P, H], F32, name="xe", tag="xe")
                nc.scalar.activation(xe[:, :], xmn[:, :], AF.Exp, scale=-1.0)
                x2 = wpool.tile([P, H], F32, name="x2", tag="x2")
                nc.vector.scalar_tensor_tensor(
                    out=x2[:, :], in0=xe[:, :], scalar=-1.0, in1=xr[:, :],
                    op0=ALU.add, op1=ALU.add)
                return x2

            # layer-1 post: x2 -> hs2 shard rows
            def post1(i, num_ps):
                x2 = finish_x(num_ps, b1rep_sb)
                xt_ps = pspool.tile([P, P], F32, name="x2t_ps", tag="tps")
                nc.tensor.transpose(xt_ps[:, :], x2[:, :], ident_sb[:, :])
                x2t = wpool.tile([P, P], F32, name="x2t", tag="x2t")
                nc.scalar.activation(x2t[:, :], xt_ps[:, :], AF.Copy)
                hs_ps = pspool.tile([P, WCOL], F32, name="hs2_ps", tag="hsps")
                nc.tensor.matmul(hs_ps[:, :], lhsT=x2t[:, :], rhs=W2e[:, :],
                                 start=True, stop=True)
                store_hs(hs_ps, hs2_sh, i)

            edge_phase(hs1_f, hs1_sh, post1)

            if analysis:
                nc.sync.dma_start(hs2_f[0:SH, :], hs2_sh[:, :])
            else:
                nc.gpsimd.collective_compute(
                    "AllGather", ALU.bypass, groups,
                    ins=[hs2_sh[:, :]], outs=[hs2_f[:, :]])

            # layer-2 post: y = x3 @ Wlin + blin; pool matmul accumulate
            pool_ps = apool.tile([G, O], F32, name="pool_ps", tag="poolps")

            def post2(i, num_ps):
                x3 = finish_x(num_ps, b2rep_sb)
                xt_ps = pspool.tile([P, P], F32, name="x3t_ps", tag="tps")
                nc.tensor.transpose(xt_ps[:, :], x3[:, :], ident_sb[:, :])
                x3t = wpool.tile([P, P], F32, name="x3t", tag="x2t")
                nc.scalar.activation(x3t[:, :], xt_ps[:, :], AF.Copy)
                y_ps = pspool.tile([P, O], F32, name="y_ps", tag="hsps")
                nc.tensor.matmul(y_ps[:, :], lhsT=x3t[:, :], rhs=Wlin_sb[:, :],
                                 start=True, stop=True)
                y_sb = wpool.tile([P, O], F32, name="y_sb", tag="y_sb")
                nc.vector.tensor_tensor(out=y_sb[:, :], in0=y_ps[:, :],
                                        in1=blinrep_sb[:, :], op=ALU.add)
                nc.tensor.matmul(pool_ps[:, :],
                                 lhsT=mgoh_sb[:, i * G:(i + 1) * G],
                                 rhs=y_sb[:, :],
                                 start=(i == 0), stop=(i == NBPC - 1))

            edge_phase(hs2_f, hs2_sh, post2)

            # ---------------- final reduce ----------------
            pool_sb = spool.tile([G, O], F32, name="pool_sb", tag="pool_sb")
            nc.scalar.activation(pool_sb[:, :], pool_ps[:, :], AF.Copy)
            nc.sync.dma_start(pool_in[:, :], pool_sb[:, :])
            if analysis:
                nc.sync.dma_start(pool_out[:, :], pool_in[:, :])
            else:
                nc.gpsimd.collective_compute(
                    "AllReduce", ALU.add, groups,
                    ins=[pool_in[:, :]], outs=[pool_out[:, :]])
            red_sb = spool.tile([G, O], F32, name="red_sb", tag="red_sb")
            nc.sync.dma_start(red_sb[:, :], pool_out[:, :])
            fin_sb = spool.tile([G, O], F32, name="fin_sb", tag="fin_sb")
            nc.vector.tensor_scalar(out=fin_sb[:, :], in0=red_sb[:, :],
                                    scalar1=cinv_sb[:, :], scalar2=None,
                                    op0=ALU.mult)
            nc.sync.dma_start(t_out.ap(), fin_sb[:, :])

    nc.compile()
    nc.m = get_hw_module(nc.m)
    return nc


_CACHE = {}


def _get_nc(meta):
    key = (meta["TOTCH"], meta["nlo"], meta["nhi"])
    if key not in _CACHE:
        _CACHE[key] = _build(meta)
    return _CACHE[key]


def run(inputs, trace=False, **kw):
    meta, percore, consts = _prep(inputs)
    nc = _get_nc(meta)
    in_maps = []
    for c in range(NCORES):
        m = dict(consts)
        m["idx_w"] = np.ascontiguousarray(percore["idx_w"][c])
        m["oh"] = np.ascontiguousarray(percore["oh"][c])
        m["mgoh"] = np.ascontiguousarray(percore["mgoh"][c])
        m["x1t"] = np.ascontiguousarray(percore["x1t"][c])
        in_maps.append(m)
    res = bass_utils.run_bass_kernel_spmd(
        nc, in_maps, core_ids=list(range(NCORES)), trace=trace, **kw)
    return res


def kernel(**inputs):
    res = run(inputs, trace=False)
    return res.results[0]["out"]


# revision 16
# speedup vs baseline: 1.3565x; 1.0813x over previous
"""Trainium2 Bass kernel for nn_EquivariantGNN_GAT (2-layer GAT + linear + mean pool).

Strategy (8 NeuronCores, SPMD single program):
  - Nodes padded to 50176 = 392 blocks of 128; each core owns 49 dst-blocks
    (6272 nodes) and all non-self edges incident (by dst) on them. The shared
    hs table rows are laid out halves-major ((half, core, local) order) so
    each AllGather can run as two half-collectives overlapped with compute.
  - Per layer, each core computes hs = x @ [W | 0 | W@a_src | W@a_dst] for its
    node shard in f32, stores the per-node row [h(128) | 1 | s_src | s_dst]
    cast to bf16 (512B rows), AllGathers the full [50176, 256] bf16 table
    into HBM (two halves), then processes its edges in chunks of 128 via
    dma_gather of hs[src] rows. Edges are src-sorted per dst block; each
    gather group of <=8 chunks uses a sliding 32768-row table view so int16
    indices suffice (no lo/hi split).
  - Per chunk: the dst one-hot matrix is HOST-PRECOMPUTED (graph-static) and
    DMA'd from HBM via static HWDGE (no DVE build, no SWDGE descgen). Scores:
    s_dst per edge via one STT accumulate against the one-hot; Pe =
    exp(leaky_relu(s_src + s_dst)) with the exp on ScalarE. The matmul rhs is
    the gathered row scaled by Pe on ScalarE (per-partition activation scale),
    so its "1" column carries Pe for the softmax denominator. lhsT is the pure
    one-hot; a single bf16 matmul accumulates numerator + denominator into
    f32 PSUM per dst block.
  - Self loops are handled analytically per block from the core's own hs rows
    (no gather), added to the PSUM result before the epilogue.
  - Softmax max-subtraction is skipped (mathematically equivalent here).
  - ELU epilogue runs on ScalarE (relu/exp) + one DVE combine, avoiding fp32
    2-port DVE ops that contend with SWDGE descriptor rings.
  - Layer-1 inputs x1 = [pos | emb[z]] and the per-graph pool one-hots are
    host-precomputed; stage A is one matmul per block.
  - Final: y = x3 @ Wlin + blin per block, per-graph mean pool via one-hot
    matmul accumulated in PSUM, AllReduce over cores, scale by 1/counts.

kernel(**inputs) takes the FULL problem inputs and returns the [64, 32] output.
"""
import sys

sys.path.insert(0, "/opt/trn_rl_repo")

import ml_dtypes
import numpy as np

import concourse.bass as bass
import concourse.bacc as bacc
import concourse.mybir as mybir
import concourse.tile as tile
import concourse.bass_utils as bass_utils
from concourse.bass_interp import get_hw_module

N = 50000
E = 1600000
H = 128
O = 32
T = 100
G = 64
P = 128
NCORES = 8
NBPC = 49              # dst blocks per core
NB = NBPC * NCORES     # 392 blocks -> 50176 padded nodes
NPAD = NB * P
SH = NBPC * P          # 6272 nodes per core
HB1 = 25 * P           # first-half local rows (blocks 0..24)
HB2 = SH - HB1         # second-half local rows (blocks 25..48)
ROW = 256              # bf16 elems per hs row: [h(128) | 1 | ssrc | sdst | 0pad]
WCOL = 131             # computed columns: [W(128) | 0 | W@a_s | W@a_d]
VIEW = 32768           # int16-addressable table window
GMAX = 8               # chunks per dma_gather (1024 descs = SWDGE ring cap)
NEG = 0.2

F32 = mybir.dt.float32
BF16 = mybir.dt.bfloat16
I16 = mybir.dt.int16
ALU = mybir.AluOpType
AF = mybir.ActivationFunctionType
NPBF = ml_dtypes.bfloat16


def _rowid(n):
    """Physical node id -> table row id (identity; AllGather concat order)."""
    return n


# ---------------------------------------------------------------- host prep
def _wrap16(flat):
    """dma_gather index layout: idx k -> [k%16, k//16], replicated x8."""
    n = flat.shape[0]
    assert n % 16 == 0
    w = flat.reshape(n // 16, 16).T          # [16, n//16]
    return np.tile(w, (8, 1))                 # [128, n//16]


def _prep(inputs):
    pos = np.ascontiguousarray(np.asarray(inputs["pos"], np.float32))
    z = np.asarray(inputs["z"]).astype(np.int64)
    ei = np.asarray(inputs["edge_index"]).astype(np.int64)
    batch = np.asarray(inputs["batch"]).astype(np.int64)
    emb = np.asarray(inputs["emb"], np.float32)

    src = ei[0]
    dst = ei[1]
    order = np.argsort(dst, kind="stable")
    src_s = src[order].astype(np.int64)
    dst_s = dst[order].astype(np.int64)

    bnd = np.searchsorted(dst_s, np.arange(NB + 1) * P).astype(np.int64)

    # per block: edges (src rowid sorted), chunked into 128s, grouped into
    # dma_gather calls of <=GMAX chunks whose rowid span fits an int16 view.
    blk_edges = {}
    for b in range(NB):
        e0, e1 = int(bnd[b]), int(bnd[b + 1])
        s = src_s[e0:e1]
        d = dst_s[e0:e1] - b * P
        srow = _rowid(s)
        o = np.argsort(srow, kind="stable")
        blk_edges[b] = (srow[o], d[o])

    # shared (max-over-cores) chunk counts per local block index
    nch_arr = np.zeros(NBPC, np.int64)
    for i in range(NBPC):
        for c in range(NCORES):
            b = NBPC * c + i
            nch_arr[i] = max(nch_arr[i], (len(blk_edges[b][0]) + P - 1) // P)
        nch_arr[i] = max(nch_arr[i], 1)
    off = np.zeros(NBPC + 1, np.int64)
    off[1:] = np.cumsum(nch_arr)
    TOTCH = int(off[-1])

    idx_w = np.zeros((NCORES, P, 8 * TOTCH), np.int16)
    oh = np.zeros((NCORES, P, TOTCH * P), NPBF)
    mgoh = np.zeros((NCORES, P, NBPC * G), np.float32)
    x1t = np.zeros((NCORES, P, SH), np.float32)

    # group structure: per local block, tuple of (sn, cb, base); shared across
    # cores, so bases must satisfy every core's rowid range for that group.
    groups_pb = []
    srows_pad = {}
    for i in range(NBPC):
        nch = int(nch_arr[i])
        for c in range(NCORES):
            b = NBPC * c + i
            srow, d = blk_edges[b]
            ne = len(srow)
            spad = np.zeros(nch * P, np.int64)
            spad[:ne] = srow
            if ne:
                spad[ne:] = srow[-1]          # pads share the last row
            else:
                # empty block: ramp tracking other cores' sorted quantiles
                spad[:] = np.linspace(0, NPAD - 1, nch * P).astype(np.int64)
            srows_pad[b] = spad
        # greedy grouping over chunks (shared): group [c0, c1) valid if for
        # every core max(srow)-min(srow) within those chunks <= VIEW-1.
        sp_all = np.stack([srows_pad[NBPC * c + i] for c in range(NCORES)])

        def span(c0, c1):
            w = sp_all[:, c0 * P:c1 * P]
            return int(w.min()), int(w.max())

        grps = []
        c0 = 0
        while c0 < nch:
            c1 = c0 + 1
            while c1 < nch and c1 - c0 < GMAX:
                lo, hi = span(c0, c1 + 1)
                if hi - lo >= VIEW:
                    break
                c1 += 1
            lo, hi = span(c0, c1)
            assert hi - lo < VIEW, f"single-chunk span too wide: blk {i} {c0}"
            base = max(hi - (VIEW - 1), 0)
            assert base <= lo and base <= NPAD - VIEW
            grps.append((c1 - c0, int(off[i]) + c0, base))
            c0 = c1
        groups_pb.append(tuple(grps))

        for c in range(NCORES):
            b = NBPC * c + i
            srow, d = blk_edges[b]
            ne = len(srow)
            for sn, cb, base in grps:
                c0 = cb - int(off[i])
                sp = srows_pad[b][c0 * P:(c0 + sn) * P]
                idx_w[c, :, 8 * cb:8 * (cb + sn)] = \
                    _wrap16((sp - base).astype(np.int16))
            if ne:
                e = np.arange(ne)
                col = (int(off[i]) + e // P) * P + d[:ne]
                oh[c, e % P, col] = 1.0

    x1_full = np.concatenate([pos, emb[z]], axis=-1).astype(np.float32)

    for c in range(NCORES):
        n0 = SH * c
        nodes = np.arange(n0, n0 + SH)
        valid = nodes < N
        bc = np.full(SH, -1, np.int64)
        bc[valid] = batch[nodes[valid]]
        for i in range(NBPC):
            blk = bc[i * P:(i + 1) * P]
            vm = blk >= 0
            mgoh[c, np.arange(P)[vm], i * G + blk[vm]] = 1.0
        x1t[c][:, valid] = x1_full[nodes[valid]].T

    counts = np.bincount(batch, minlength=G).astype(np.float32)
    cinv = (1.0 / np.maximum(counts, 1.0)).astype(np.float32).reshape(G, 1)

    consts = dict(
        ones1=np.ones((1, P), NPBF),
        ident=np.eye(P, dtype=np.float32),
        W1=np.ascontiguousarray(np.asarray(inputs["W1"], np.float32)),
        W1T=np.ascontiguousarray(np.asarray(inputs["W1"], np.float32).T),
        a1s=np.asarray(inputs["a1_src"], np.float32).reshape(H, 1),
        a1d=np.asarray(inputs["a1_dst"], np.float32).reshape(H, 1),
        b1rep=np.ascontiguousarray(
            np.tile(np.asarray(inputs["b1"], np.float32), (P, 1))),
        W2=np.ascontiguousarray(np.asarray(inputs["W2"], np.float32)),
        W2T=np.ascontiguousarray(np.asarray(inputs["W2"], np.float32).T),
        a2s=np.asarray(inputs["a2_src"], np.float32).reshape(H, 1),
        a2d=np.asarray(inputs["a2_dst"], np.float32).reshape(H, 1),
        b2rep=np.ascontiguousarray(
            np.tile(np.asarray(inputs["b2"], np.float32), (P, 1))),
        Wlin=np.ascontiguousarray(np.asarray(inputs["Wlin"], np.float32)),
        blinrep=np.ascontiguousarray(
            np.tile(np.asarray(inputs["blin"], np.float32), (P, 1))),
        cinv=cinv,
    )

    meta = dict(groups=tuple(groups_pb), off=tuple(int(x) for x in off),
                TOTCH=TOTCH)
    percore = dict(idx_w=idx_w, oh=oh, mgoh=mgoh, x1t=x1t)
    return meta, percore, consts


# ---------------------------------------------------------------- program
def _build(meta, analysis=False, nqueues=2, skip=(),
           scratch=16384, gbufs=3, bufboost=0, sp=1, pegdve=0):
    groups_pb = meta["groups"]
    TOTCH = meta["TOTCH"]
    qstate = {"q": 0}

    def next_q():
        q = qstate["q"]
        qstate["q"] = (q + 1) % nqueues
        return q

    nc = bacc.Bacc("TRN2", target_bir_lowering=False, debug=False,
                   enable_asserts=False,
                   num_devices=1 if analysis else NCORES,
                   num_swdge_queues=nqueues,
                   dynamic_dma_scratch_size=scratch)

    t_idx = nc.dram_tensor("idx_w", [P, 8 * TOTCH], I16, kind="ExternalInput")
    t_oh = nc.dram_tensor("oh", [P, TOTCH * P], BF16, kind="ExternalInput")
    t_mgoh = nc.dram_tensor("mgoh", [P, NBPC * G], F32, kind="ExternalInput")
    t_x1t = nc.dram_tensor("x1t", [P, SH], F32, kind="ExternalInput")
    t_ones1 = nc.dram_tensor("ones1", [1, P], BF16, kind="ExternalInput")
    t_ident = nc.dram_tensor("ident", [P, P], F32, kind="ExternalInput")
    t_W1 = nc.dram_tensor("W1", [H, H], F32, kind="ExternalInput")
    t_W1T = nc.dram_tensor("W1T", [H, H], F32, kind="ExternalInput")
    t_a1s = nc.dram_tensor("a1s", [H, 1], F32, kind="ExternalInput")
    t_a1d = nc.dram_tensor("a1d", [H, 1], F32, kind="ExternalInput")
    t_b1rep = nc.dram_tensor("b1rep", [P, H], F32, kind="ExternalInput")
    t_W2 = nc.dram_tensor("W2", [H, H], F32, kind="ExternalInput")
    t_W2T = nc.dram_tensor("W2T", [H, H], F32, kind="ExternalInput")
    t_a2s = nc.dram_tensor("a2s", [H, 1], F32, kind="ExternalInput")
    t_a2d = nc.dram_tensor("a2d", [H, 1], F32, kind="ExternalInput")
    t_b2rep = nc.dram_tensor("b2rep", [P, H], F32, kind="ExternalInput")
    t_Wlin = nc.dram_tensor("Wlin", [H, O], F32, kind="ExternalInput")
    t_blinrep = nc.dram_tensor("blinrep", [P, O], F32, kind="ExternalInput")
    t_cinv = nc.dram_tensor("cinv", [G, 1], F32, kind="ExternalInput")
    t_out = nc.dram_tensor("out", [G, O], F32, kind="ExternalOutput")

    groups = [list(range(NCORES))]

    with tile.TileContext(nc) as tc:
        with (
            tc.tile_pool(name="const", bufs=1) as cpool,
            tc.tile_pool(name="work", bufs=3) as wpool,
            tc.tile_pool(name="gat", bufs=gbufs) as gpool,
            tc.tile_pool(name="ohp", bufs=gbufs) as ohpool,
            tc.tile_pool(name="mt", bufs=14 + bufboost) as mpool,
            tc.tile_pool(name="sml", bufs=4 + bufboost) as spool,
            tc.tile_pool(name="ps", bufs=2, space="PSUM") as pspool,
            tc.tile_pool(name="acc", bufs=1, space="PSUM") as apool,
            tc.tile_pool(name="dram", bufs=1, space="DRAM") as dpool,
        ):
            def cload(nm, t, shape, dtype=F32):
                tl = cpool.tile(shape, dtype, name=nm, tag=nm)
                nc.sync.dma_start(tl[:, :], t.ap())
                return tl

            ones1_sb = cload("ones1_sb", t_ones1, [1, P], BF16)
            ident_sb = cload("ident_sb", t_ident, [P, P])
            b1rep_sb = cload("b1rep_sb", t_b1rep, [P, H])
            b2rep_sb = cload("b2rep_sb", t_b2rep, [P, H])
            Wlin_sb = cload("Wlin_sb", t_Wlin, [H, O])
            blinrep_sb = cload("blinrep_sb", t_blinrep, [P, O])
            cinv_sb = cload("cinv_sb", t_cinv, [G, 1])
            idx_sb = cload("idx_sb", t_idx, [P, 8 * TOTCH], I16)
            mgoh_sb = cload("mgoh_sb", t_mgoh, [P, NBPC * G])

            # folded weight tables We = [W | 0 | W@a_s | W@a_d]
            def fold(nm, tW, tWT, tas, tad):
                We = cpool.tile([H, WCOL], F32, name=nm, tag=nm)
                nc.sync.dma_start(We[:, 0:H], tW.ap())
                nc.vector.memset(We[:, 128:129], 0.0)
                WT_sb = cpool.tile([H, H], F32, name=nm + "_WT", tag=nm + "_WT")
                nc.sync.dma_start(WT_sb[:, :], tWT.ap())
                for col, tvec in ((129, tas), (130, tad)):
                    av = cpool.tile([H, 1], F32, name=f"{nm}_a{col}",
                                    tag=f"{nm}_a{col}")
                    nc.sync.dma_start(av[:, :], tvec.ap())
                    fps = pspool.tile([H, 1], F32, name=f"{nm}_f{col}", tag="tps")
                    nc.tensor.matmul(fps[:, :], lhsT=WT_sb[:, :], rhs=av[:, :],
                                     start=True, stop=True)
                    nc.scalar.activation(We[:, col:col + 1], fps[:, :], AF.Copy)
                return We

            W1e = fold("W1e", t_W1, t_W1T, t_a1s, t_a1d)
            W2e = fold("W2e", t_W2, t_W2T, t_a2s, t_a2d)

            hs1_sh = dpool.tile([SH, ROW], BF16, name="hs1_sh", tag="hs1_sh")
            hs1_f = dpool.tile([NPAD, ROW], BF16, name="hs1_f", tag="hs1_f",
                               addr_space="Shared")
            hs2_sh = dpool.tile([SH, ROW], BF16, name="hs2_sh", tag="hs2_sh")
            hs2_f = dpool.tile([NPAD, ROW], BF16, name="hs2_f", tag="hs2_f",
                               addr_space="Shared")
            pool_in = dpool.tile([G, O], F32, name="pool_in", tag="pool_in")
            pool_out = dpool.tile([G, O], F32, name="pool_out", tag="pool_out",
                                  addr_space="Shared")

            def full_gather(hs_sh_d, hs_f_d):
                if analysis:
                    nc.sync.dma_start(hs_f_d[0:SH, :], hs_sh_d[:, :])
                else:
                    nc.gpsimd.collective_compute(
                        "AllGather", ALU.bypass, groups,
                        ins=[hs_sh_d[:, :]], outs=[hs_f_d[:, :]])

            # store [h | 1 | ssrc | sdst] (f32 psum) as bf16 row, zero pad
            def store_hs(hs_ps, hs_dram, i):
                hs_sb = wpool.tile([P, ROW], BF16, name="hs_sb", tag="hs_sb")
                nc.scalar.activation(hs_sb[:, 0:WCOL], hs_ps[:, :], AF.Copy)
                nc.vector.memset(hs_sb[:, 128:129], 1.0)
                nc.vector.memset(hs_sb[:, WCOL:ROW], 0.0)
                nc.sync.dma_start(hs_dram[i * P:(i + 1) * P, :], hs_sb[:, :])

            # ---------------- stage A: hs1 for own shard ----------------
            for i in range(NBPC):
                x1ts = wpool.tile([P, P], F32, name="x1ts", tag="x1ts")
                nc.sync.dma_start(x1ts[:, :], t_x1t.ap()[:, i * P:(i + 1) * P])
                hs_ps = pspool.tile([P, WCOL], F32, name="hs_ps", tag="hsps")
                nc.tensor.matmul(hs_ps[:, :], lhsT=x1ts[:, :], rhs=W1e[:, :],
                                 start=True, stop=True)
                store_hs(hs_ps, hs1_sh, i)
            full_gather(hs1_sh, hs1_f)

            # ---------------- edge phase ----------------
            def edge_phase(hs_f, hs_sh_d, post_block, mid_hook=None):
                for i in range(NBPC):
                    # block prep: s_dst row replicated across partitions
                    sd_row = spool.tile([1, P], BF16, name="sd_row",
                                        tag="sd_row")
                    nc.sync.dma_start(
                        sd_row[:, :],
                        hs_sh_d[i * P:(i + 1) * P, 130:131].transpose([1, 0]))
                    sd_ps = pspool.tile([P, P], F32, name="sd_ps", tag="tps")
                    nc.tensor.matmul(sd_ps[:, :], lhsT=ones1_sb[:, :],
                                     rhs=sd_row[:, :], start=True, stop=True)
                    sdst_rep = wpool.tile([P, P], BF16, name="sdst_rep",
                                          tag="sdst_rep")
                    nc.scalar.activation(sdst_rep[:, :], sd_ps[:, :], AF.Copy)

                    # analytic self-loop term from own rows (no gather)
                    own = wpool.tile([P, WCOL], BF16, name="own", tag="own")
                    nc.sync.dma_start(own[:, :],
                                      hs_sh_d[i * P:(i + 1) * P, 0:WCOL])
                    qs = spool.tile([P, 1], F32, name="qs", tag="qs")
                    nc.vector.tensor_tensor(out=qs[:, :],
                                            in0=own[:, 129:130],
                                            in1=own[:, 130:131], op=ALU.add)
                    vs = spool.tile([P, 1], F32, name="vs", tag="vs")
                    nc.vector.scalar_tensor_tensor(
                        out=vs[:, :], in0=qs[:, :], scalar=NEG,
                        in1=qs[:, :], op0=ALU.mult, op1=ALU.max)
                    pes = spool.tile([P, 1], F32, name="pes", tag="pes")
                    nc.scalar.activation(pes[:, :], vs[:, :], AF.Exp)
                    selft = wpool.tile([P, 129], F32, name="selft", tag="selft")
                    nc.scalar.mul(selft[:, :], own[:, 0:129], pes[:, :])

                    num_ps = pspool.tile([P, 129], F32, name="num_ps",
                                         tag="numps")

                    grps = groups_pb[i]
                    nch = sum(g[0] for g in grps)
                    # emit all gathers for this block first (prefetch)
                    work = []
                    done = 0
                    for sn, cb, base in grps:
                        Gt = gpool.tile([P, sn * ROW], BF16, name="Gt",
                                        tag="Gt")
                        nc.gpsimd.dma_gather(
                            out_ap=Gt.rearrange("p (c s) -> p c s", s=ROW),
                            in_ap=hs_f[base:base + VIEW, :],
                            idxs_ap=idx_sb[:, 8 * cb:8 * (cb + sn)],
                            num_idxs=sn * P,
                            num_idxs_reg=sn * P,
                            elem_size=ROW,
                            single_packet=bool(sp),
                            queue_num=next_q(),
                        )
                        ohg = ohpool.tile([P, sn * P], BF16, name="ohg",
                                          tag="ohg")
                        nc.sync.dma_start(
                            ohg[:, :],
                            t_oh.ap()[:, P * cb:P * (cb + sn)])
                        work.append((Gt, ohg, sn, cb, done))
                        done += sn
                    for Gt, ohg, sn, cb, base in work:
                        SD = spool.tile([P, GMAX], F32, name="SD", tag="SD")
                        for jj in range(sn):
                            junk = mpool.tile([P, P], BF16, name="junk",
                                              tag="junk")
                            nc.vector.scalar_tensor_tensor(
                                out=junk[:, :],
                                in0=ohg[:, jj * P:(jj + 1) * P],
                                scalar=1.0,
                                in1=sdst_rep[:, :],
                                op0=ALU.mult, op1=ALU.mult,
                                accum_out=SD[:, jj:jj + 1])
                        ssrc = Gt.rearrange("p (c s) -> p c s",
                                            s=ROW)[:, :, 129:130].squeeze(2)
                        Q = spool.tile([P, GMAX], F32, name="Q", tag="Q")
                        nc.vector.tensor_tensor(out=Q[:, 0:sn],
                                                in0=SD[:, 0:sn],
                                                in1=ssrc, op=ALU.add)
                        V = spool.tile([P, GMAX], F32, name="V", tag="V")
                        nc.vector.scalar_tensor_tensor(
                            out=V[:, 0:sn], in0=Q[:, 0:sn], scalar=NEG,
                            in1=Q[:, 0:sn], op0=ALU.mult, op1=ALU.max)
                        Pe = spool.tile([P, GMAX], F32, name="Pe", tag="Pe")
                        nc.scalar.activation(Pe[:, 0:sn], V[:, 0:sn], AF.Exp)
                        for jj in range(sn):
                            peg = mpool.tile([P, 129], BF16, name="peg",
                                             tag="peg")
                            if pegdve and jj % 2 == 0:
                                nc.vector.tensor_scalar(
                                    out=peg[:, :],
                                    in0=Gt[:, jj * ROW:jj * ROW + 129],
                                    scalar1=Pe[:, jj:jj + 1], scalar2=None,
                                    op0=ALU.mult)
                            else:
                                nc.scalar.mul(peg[:, :],
                                              Gt[:, jj * ROW:jj * ROW + 129],
                                              Pe[:, jj:jj + 1])
                            if "mm" not in skip or base + jj == 0:
                                nc.tensor.matmul(
                                    num_ps[:, :],
                                    lhsT=ohg[:, jj * P:(jj + 1) * P],
                                    rhs=peg[:, :],
                                    start=(base + jj == 0),
                                    stop=(base + jj == nch - 1)
                                    if "mm" not in skip else True)
                    post_block(i, num_ps, selft)
                    if mid_hook is not None and i == 24:
                        mid_hook()

            # common post-block epilogue: x = elu(num/den + b)
            # elu(x) = relu(x) + exp(min(x, 0)) - 1; min(x,0) = -relu(-x).
            # relu/exp on ScalarE (fp32 2-port DVE ops contend with SWDGE
            # descriptor rings), one DVE combine.
            def finish_x(num_ps, selft, brep_sb):
                numt = wpool.tile([P, 129], F32, name="numt", tag="numt")
                nc.vector.tensor_tensor(out=numt[:, :], in0=num_ps[:, :],
                                        in1=selft[:, :], op=ALU.add)
                den = spool.tile([P, 1], F32, name="den", tag="den")
                nc.vector.tensor_scalar(out=den[:, :], in0=numt[:, 128:129],
                                        scalar1=1e-30, scalar2=None, op0=ALU.max)
                rec = spool.tile([P, 1], F32, name="rec", tag="rec")
                nc.vector.reciprocal(rec[:, :], den[:, :])
                xp = wpool.tile([P, H], F32, name="xp", tag="xp")
                nc.vector.scalar_tensor_tensor(
                    out=xp[:, :], in0=numt[:, 0:128], scalar=rec[:, :],
                    in1=brep_sb[:, :], op0=ALU.mult, op1=ALU.add)
                xr = wpool.tile([P, H], F32, name="xr", tag="xr")
                nc.scalar.activation(xr[:, :], xp[:, :], AF.Relu)
                xmn = wpool.tile([P, H], F32, name="xmn", tag="xmn")
                nc.scalar.activation(xmn[:, :], xp[:, :], AF.Relu, scale=-1.0)
                xe = wpool.tile([P, H], F32, name="xe", tag="xe")
                nc.scalar.activation(xe[:, :], xmn[:, :], AF.Exp, scale=-1.0)
                x2 = wpool.tile([P, H], F32, name="x2", tag="x2")
                nc.vector.scalar_tensor_tensor(
                    out=x2[:, :], in0=xe[:, :], scalar=-1.0, in1=xr[:, :],
                    op0=ALU.add, op1=ALU.add)
                return x2

            # layer-1 post: x2 -> hs2 shard rows
            def post1(i, num_ps, selft):
                x2 = finish_x(num_ps, selft, b1rep_sb)
                xt_ps = pspool.tile([P, P], F32, name="x2t_ps", tag="tps")
                nc.tensor.transpose(xt_ps[:, :], x2[:, :], ident_sb[:, :])
                x2t = wpool.tile([P, P], F32, name="x2t", tag="x2t")
                nc.scalar.activation(x2t[:, :], xt_ps[:, :], AF.Copy)
                hs_ps = pspool.tile([P, WCOL], F32, name="hs2_ps", tag="hsps")
                nc.tensor.matmul(hs_ps[:, :], lhsT=x2t[:, :], rhs=W2e[:, :],
                                 start=True, stop=True)
                store_hs(hs_ps, hs2_sh, i)

            edge_phase(hs1_f, hs1_sh, post1)
            full_gather(hs2_sh, hs2_f)

            # layer-2 post: y = x3 @ Wlin + blin; pool matmul accumulate
            pool_ps = apool.tile([G, O], F32, name="pool_ps", tag="poolps")

            def post2(i, num_ps, selft):
                x3 = finish_x(num_ps, selft, b2rep_sb)
                xt_ps = pspool.tile([P, P], F32, name="x3t_ps", tag="tps")
                nc.tensor.transpose(xt_ps[:, :], x3[:, :], ident_sb[:, :])
                x3t = wpool.tile([P, P], F32, name="x3t", tag="x2t")
                nc.scalar.activation(x3t[:, :], xt_ps[:, :], AF.Copy)
                y_ps = pspool.tile([P, O], F32, name="y_ps", tag="hsps")
                nc.tensor.matmul(y_ps[:, :], lhsT=x3t[:, :], rhs=Wlin_sb[:, :],
                                 start=True, stop=True)
                y_sb = wpool.tile([P, O], F32, name="y_sb", tag="y_sb")
                nc.vector.tensor_tensor(out=y_sb[:, :], in0=y_ps[:, :],
                                        in1=blinrep_sb[:, :], op=ALU.add)
                nc.tensor.matmul(pool_ps[:, :],
                                 lhsT=mgoh_sb[:, i * G:(i + 1) * G],
                                 rhs=y_sb[:, :],
                                 start=(i == 0), stop=(i == NBPC - 1))

            edge_phase(hs2_f, hs2_sh, post2)

            # ---------------- final reduce ----------------
            pool_sb = spool.tile([G, O], F32, name="pool_sb", tag="pool_sb")
            nc.scalar.activation(pool_sb[:, :], pool_ps[:, :], AF.Copy)
            nc.sync.dma_start(pool_in[:, :], pool_sb[:, :])
            if analysis:
                nc.sync.dma_start(pool_out[:, :], pool_in[:, :])
            else:
                nc.gpsimd.collective_compute(
                    "AllReduce", ALU.add, groups,
                    ins=[pool_in[:, :]], outs=[pool_out[:, :]])
            red_sb = spool.tile([G, O], F32, name="red_sb", tag="red_sb")
            nc.sync.dma_start(red_sb[:, :], pool_out[:, :])
            fin_sb = spool.tile([G, O], F32, name="fin_sb", tag="fin_sb")
            nc.vector.tensor_scalar(out=fin_sb[:, :], in0=red_sb[:, :],
                                    scalar1=cinv_sb[:, :], scalar2=None,
                                    op0=ALU.mult)
            nc.sync.dma_start(t_out.ap(), fin_sb[:, :])

    nc.compile()
    nc.m = get_hw_module(nc.m)
    return nc


_CACHE = {}


def _get_nc(meta):
    key = (meta["TOTCH"], meta["groups"])
    if key not in _CACHE:
        _CACHE[key] = _build(meta)
    return _CACHE[key]


def run(inputs, trace=False, **kw):
    meta, percore, consts = _prep(inputs)
    nc = _get_nc(meta)
    in_maps = []
    for c in range(NCORES):
        m = dict(consts)
        m["idx_w"] = np.ascontiguousarray(percore["idx_w"][c])
        m["oh"] = np.ascontiguousarray(percore["oh"][c])
        m["mgoh"] = np.ascontiguousarray(percore["mgoh"][c])
        m["x1t"] = np.ascontiguousarray(percore["x1t"][c])
        in_maps.append(m)
    res = bass_utils.run_bass_kernel_spmd(
        nc, in_maps, core_ids=list(range(NCORES)), trace=trace, **kw)
    return res


def kernel(**inputs):
    res = run(inputs, trace=False)
    return res.results[0]["out"]
